# revision 1
# baseline (speedup 1.0000x reference)
"""Depthwise-separable conv2d block (dw3x3 + BN + ReLU + map-cut, pw1x1 + BN +
ReLU + map-cut) on 8 Trainium2 NeuronCores, data-parallel over the batch dim.

Fixed problem shapes: x (32,256,56,56) f32 -> out (32,512,54,54) f32.

Per-core device program (4 images each, fp8 e4m3 matmul operands in DoubleRow
perf mode = 2 contraction rows per cycle, f32 PSUM):
  - depthwise 3x3 VALID conv: the 9 taps are packed into 5 DoubleRow tap-PAIR
    matmuls per chunk (vs 9 plain matmuls).  Each chunk streams 9 contiguous
    56-wide input rows flat (504 cols incl. 2 junk cols per row that wrap the
    row boundary); tap pairs are overlapping strided views of the same rows.
  - chunk max is reduced from PSUM (junk cols excluded via a [9,54] view);
    ACT drains relu(conv+bias) straight to an fp8 Y tile (scaled x16)
  - the per-(image,channel) depthwise cut mask is folded into per-image
    masked copies of the pointwise weights (w2 * mask)
  - pointwise 1x1 conv: ONE DoubleRow matmul per 486-col chunk contracts all
    256 input channels; ACT drains (PSUM/32+bias) -> f32 Z per chunk (PSUM
    frees early) while DVE reduces per-chunk maxes; after chunk 5 the cut
    mask gates a relu*mask pass per half, stores overlap the DMA
BatchNorm (inference) is folded into the conv weights/biases on the host.
"""

import ml_dtypes
import numpy as np

import concourse.bacc as bacc
import concourse.bass as bass
import concourse.mybir as mybir
import concourse.tile as tile
from concourse.bass_utils import run_bass_kernel_spmd

EPS = 1e-5
DW_THRESH = 4.0
PW_THRESH = 0.001

B, CIN, COUT, H, W = 32, 256, 512, 56, 56
HO, WO = 54, 54
NPIX = HO * WO          # 2916
NCORES = 8
BPC = B // NCORES       # 4 images per core
P = 128                 # partitions
KT = CIN // P           # 2 cin tiles
MT = COUT // P          # 4 cout tiles
NCH = 6                 # output chunks per map: 6 x (9 rows x 54 cols)
CHROWS = HO // NCH      # 9
CHUNK = CHROWS * WO     # 486 valid columns per chunk
FCH = CHROWS * W        # 504 flat columns per chunk (incl. 2 junk cols/row)
XLEN = H * W            # 3136
XPAD = XLEN + 3         # pad so the last chunk's shifted taps stay in bounds
NEG = -3.0e38

S1 = 16.0               # dw weight scale (fp8); Y holds y (descaled at drain)
S2 = 32.0               # pw weight scale (fp8): PSUM2 holds 32*z_conv
SINV = 1.0 / S2

F32 = mybir.dt.float32
FP8 = mybir.dt.float8e4
NP8 = ml_dtypes.float8_e4m3

# tap pairs for DoubleRow.  HW constraint: the pair (dim-1) stride of the
# moving AP must be EVEN, so taps are paired (dj, dj+2) within a row
# (stride 2), plus (t1,t7) at stride 112 and t4 with a zero-weight slot.
PAIRS = [(0, 2), (3, 5), (6, 8), (1, 7), (4, None)]


def _tap_off(n, t):
    # flat offset of tap t's first element for chunk n (out rows 9n..9n+8)
    return (CHROWS * n + t // 3) * W + t % 3

_cached_nc = None


def _build_program():
    nc = bacc.Bacc("TRN2", target_bir_lowering=False, debug=False)

    xs = nc.dram_tensor("xs", [BPC, CIN, XLEN], FP8, kind="ExternalInput").ap()
    dwp = nc.dram_tensor("dwp", [P, KT, 5, 2, P], FP8, kind="ExternalInput").ap()
    w2t = nc.dram_tensor("w2t", [P, KT, COUT], FP8, kind="ExternalInput").ap()
    b1s = nc.dram_tensor("b1s", [P, KT], F32, kind="ExternalInput").ap()
    t1s = nc.dram_tensor("t1s", [P, KT], F32, kind="ExternalInput").ap()
    b2s = nc.dram_tensor("b2s", [P, MT], F32, kind="ExternalInput").ap()
    t2s = nc.dram_tensor("t2s", [P, MT], F32, kind="ExternalInput").ap()
    zs = nc.dram_tensor("zs", [BPC, COUT, NPIX], F32, kind="ExternalOutput").ap()

    with tile.TileContext(nc) as tc:
        with (
            tc.tile_pool(name="consts", bufs=1) as consts,
            tc.tile_pool(name="xp", bufs=8) as xp,
            tc.tile_pool(name="yp", bufs=3) as yp,
            tc.tile_pool(name="zp", bufs=4) as zp,
            tc.tile_pool(name="wm", bufs=2) as wmp,
            tc.tile_pool(name="st", bufs=24) as st,
            tc.tile_pool(name="psdw", bufs=3, space="PSUM") as psdw,
            tc.tile_pool(name="pspw", bufs=3, space="PSUM") as pspw,
        ):
            # dw weights + consts first (small, gate the first matmul),
            # then the x tiles
            dwsb = consts.tile([P, KT, 5, 2, P], FP8)
            nc.sync.dma_start(out=dwsb[:, 0], in_=dwp[:, 0])
            nc.sync.dma_start(out=dwsb[:, 1], in_=dwp[:, 1])
            b1sb = consts.tile([P, KT], F32)
            nc.sync.dma_start(out=b1sb, in_=b1s)
            t1sb = consts.tile([P, KT], F32)
            nc.sync.dma_start(out=t1sb, in_=t1s)
            w2sb = consts.tile([P, KT, COUT], FP8)
            nc.sync.dma_start(out=w2sb, in_=w2t)
            b2sb = consts.tile([P, MT], F32)
            nc.sync.dma_start(out=b2sb, in_=b2s)
            t2sb = consts.tile([P, MT], F32)
            nc.sync.dma_start(out=t2sb, in_=t2s)
            xtiles = {}
            for b in range(BPC):
                for k in range(KT):
                    X = xp.tile([P, XPAD], FP8, name="X")
                    nc.sync.dma_start(out=X[:, 0:XLEN],
                                      in_=xs[b, k * P:(k + 1) * P, :])
                    nc.gpsimd.memset(X[:, XLEN:XPAD], 0)
                    xtiles[b, k] = X

            def emit_dw(b, Y, w2m):
                X0 = xtiles[b, 0]
                for k in range(KT):
                    X = xtiles[b, k]
                    mzx = st.tile([P, NCH], F32, name="mzx1")
                    for n in range(NCH):
                        P1 = psdw.tile([P, FCH], F32, name="P1")
                        for i, (ta, tb) in enumerate(PAIRS):
                            off = _tap_off(n, ta)
                            dlt = (_tap_off(n, tb) - off) if tb is not None else 2
                            rhs = bass.AP(
                                tensor=X.tensor,
                                offset=X.offset + off,
                                ap=[X.ap[0], [dlt, 2], [1, FCH]],
                            )
                            nc.tensor.matmul(
                                P1,
                                lhsT=dwsb[:, k, i],
                                rhs=rhs,
                                start=(i == 0),
                                stop=(i == 4),
                                perf_mode=mybir.MatmulPerfMode.DoubleRow,
                            )
                        # [9,54] view skips the 2 junk cols per row
                        P1v = P1.rearrange("p (r w) -> p r w", w=W)[:, :, 0:WO]
                        nc.scalar.activation(
                            out=Y[:, k, n].rearrange("p (r w) -> p r w", w=WO),
                            in_=P1v,
                            func=mybir.ActivationFunctionType.Relu,
                            bias=b1sb[:, k:k + 1], scale=1.0 / S1)
                        nc.vector.tensor_reduce(
                            mzx[:, n:n + 1], P1v,
                            axis=mybir.AxisListType.XY, op=mybir.AluOpType.max)
                    m1 = st.tile([P, 1], F32, name="m1")
                    nc.vector.tensor_reduce(
                        m1, mzx, axis=mybir.AxisListType.X,
                        op=mybir.AluOpType.max)
                    mask1 = st.tile([P, 1], F32, name="mask1")
                    nc.vector.tensor_scalar(
                        out=mask1, in0=m1, scalar1=t1sb[:, k:k + 1],
                        scalar2=None, op0=mybir.AluOpType.is_ge)
                    nc.vector.tensor_scalar(
                        out=w2m[:, k], in0=w2sb[:, k], scalar1=mask1,
                        scalar2=None, op0=mybir.AluOpType.mult)

            def emit_pw(b, Y, w2m):
                for m in range(MT):
                    Z = zp.tile([P, NPIX], F32, name="Z")
                    mzx = st.tile([P, NCH], F32, name="mzx2")
                    for n in range(NCH):
                        P2 = pspw.tile([P, CHUNK], F32, name="P2")
                        nc.tensor.matmul(
                            P2,
                            lhsT=w2m[:, :, m * P:(m + 1) * P],
                            rhs=Y[:, :, n],
                            start=True, stop=True,
                            perf_mode=mybir.MatmulPerfMode.DoubleRow,
                        )
                        # per-chunk max off PSUM (mask input), then drain
                        # (PSUM/32 + b2) -> Z; relu+mask applied in pass 2
                        nc.vector.tensor_reduce(
                            mzx[:, n:n + 1], P2,
                            axis=mybir.AxisListType.X, op=mybir.AluOpType.max)
                        nc.scalar.activation(
                            out=Z[:, n * CHUNK:(n + 1) * CHUNK], in_=P2,
                            func=mybir.ActivationFunctionType.Identity,
                            bias=b2sb[:, m:m + 1], scale=SINV)
                    m2 = st.tile([P, 1], F32, name="m2")
                    nc.vector.tensor_reduce(
                        m2, mzx, axis=mybir.AxisListType.X,
                        op=mybir.AluOpType.max)
                    mask2 = st.tile([P, 1], F32, name="mask2")
                    nc.vector.tensor_scalar(
                        out=mask2, in0=m2, scalar1=t2sb[:, m:m + 1],
                        scalar2=None, op0=mybir.AluOpType.is_ge)
                    # z = max(z*mask, 0): split halves so the store overlaps,
                    # alternating DVE / ACT to balance engines
                    HP = NPIX // 2
                    for h in range(2):
                        zslice = Z[:, h * HP:(h + 1) * HP]
                        if (m + h) % 2 == 0:
                            nc.vector.tensor_scalar(
                                out=zslice, in0=zslice, scalar1=mask2,
                                scalar2=0.0, op0=mybir.AluOpType.mult,
                                op1=mybir.AluOpType.max)
                        else:
                            nc.scalar.activation(
                                out=zslice, in_=zslice,
                                func=mybir.ActivationFunctionType.Relu,
                                bias=0.0, scale=mask2)
                        nc.sync.dma_start(
                            out=zs[b, m * P:(m + 1) * P, h * HP:(h + 1) * HP],
                            in_=zslice)

            # software pipeline: dw(b+1) runs on tensor between dw(b) and
            # pw(b) so the mask1->w2m tail never stalls the tensor queue
            ys, wms = {}, {}
            for b in range(BPC):
                ys[b] = yp.tile([P, KT, NCH, CHUNK], FP8, name="Y")
                wms[b] = wmp.tile([P, KT, COUT], FP8, name="w2m")
                emit_dw(b, ys[b], wms[b])
                if b > 0:
                    emit_pw(b - 1, ys[b - 1], wms[b - 1])
            emit_pw(BPC - 1, ys[BPC - 1], wms[BPC - 1])
    nc.compile()
    return nc


def _prep_params(dw_w, dw_b, dw_gamma, dw_beta, dw_mean, dw_var,
                 pw_w, pw_b, pw_gamma, pw_beta, pw_mean, pw_var):
    dw_scale = dw_gamma / np.sqrt(dw_var + EPS)
    b1 = dw_b * dw_scale + dw_beta - dw_mean * dw_scale          # (256,)
    w1 = dw_w[:, 0] * dw_scale[:, None, None]                    # (256,3,3)
    w1f = (S1 * w1).reshape(CIN, 9)                              # scaled taps

    dwp = np.zeros((P, KT, 5, 2, P), np.float32)
    idx = np.arange(P)
    for k in range(KT):
        for i, (ta, tb) in enumerate(PAIRS):
            dwp[idx, k, i, 0, idx] = w1f[k * P:(k + 1) * P, ta]
            if tb is not None:
                dwp[idx, k, i, 1, idx] = w1f[k * P:(k + 1) * P, tb]

    pw_scale = pw_gamma / np.sqrt(pw_var + EPS)
    b2 = pw_b * pw_scale + pw_beta - pw_mean * pw_scale          # (512,)
    w2 = pw_w * pw_scale[:, None]                                # (512,256)
    # w2t[ck, k, o] = S2 * w2[o, k*128+ck]
    w2t = np.ascontiguousarray(
        (S2 * w2).T.reshape(KT, P, COUT).transpose(1, 0, 2))
    b1s = np.ascontiguousarray(b1.reshape(KT, P).T)              # dw bias
    t1s = np.ascontiguousarray(
        S1 * (DW_THRESH - b1.reshape(KT, P).T))                  # dw cut thresh
    b2s = np.ascontiguousarray(b2.reshape(MT, P).T)              # pw bias
    t2s = np.ascontiguousarray(
        S2 * (PW_THRESH - b2.reshape(MT, P).T))                  # pw cut thresh

    def to8(a):
        return np.clip(a, -240.0, 240.0).astype(NP8)

    return (to8(dwp), to8(w2t), b1s.astype(np.float32),
            t1s.astype(np.float32), b2s.astype(np.float32),
            t2s.astype(np.float32))


def _prep_in_maps(x, dw_w, dw_b, dw_gamma, dw_beta, dw_mean, dw_var,
                  pw_w, pw_b, pw_gamma, pw_beta, pw_mean, pw_var):
    x = np.ascontiguousarray(np.asarray(x, np.float32)).reshape(B, CIN, XLEN)
    args = [np.asarray(a, np.float32) for a in
            (dw_w, dw_b, dw_gamma, dw_beta, dw_mean, dw_var,
             pw_w, pw_b, pw_gamma, pw_beta, pw_mean, pw_var)]
    dwp8, w2t8, b1s, t1s, b2s, t2s = _prep_params(*args)
    x8 = np.clip(x, -240.0, 240.0).astype(NP8)

    in_maps = []
    for c in range(NCORES):
        in_maps.append({
            "xs": np.ascontiguousarray(x8[c * BPC:(c + 1) * BPC]),
            "dwp": dwp8,
            "w2t": w2t8,
            "b1s": b1s,
            "t1s": t1s,
            "b2s": b2s,
            "t2s": t2s,
        })
    return in_maps


def kernel(x, dw_w, dw_b, dw_gamma, dw_beta, dw_mean, dw_var,
           pw_w, pw_b, pw_gamma, pw_beta, pw_mean, pw_var):
    global _cached_nc
    in_maps = _prep_in_maps(x, dw_w, dw_b, dw_gamma, dw_beta, dw_mean, dw_var,
                            pw_w, pw_b, pw_gamma, pw_beta, pw_mean, pw_var)

    if _cached_nc is None:
        _cached_nc = _build_program()
    nc = _cached_nc

    res = run_bass_kernel_spmd(nc, in_maps, core_ids=list(range(NCORES)))
    out = np.concatenate(
        [res.results[c]["zs"].reshape(BPC, COUT, HO, WO)
         for c in range(NCORES)], axis=0)
    return out

